# revision 18
# baseline (speedup 1.0000x reference)
"""DMMR loss kernel for Trainium2 (8 NeuronCores, data-parallel over patches).

Reference semantics (see problem):
  fp = extract_patches(fixed)   # [3375, 4913]
  mp = extract_patches(moving)  # [3375, 4913]
  keep = (mean(fp == 0, axis=1) <= 0.15)
  out  = tanh(sum((fp @ Wf) * (mp @ Wm), -1))  # [3375]
  value = sum(out * keep) / max(sum(keep), 1)

Sharding: the 3375 patch pairs are split 422-per-core across 8 cores and
padded to 432 columns (16-aligned for the DoubleRow moving AP).  The keep
mask is applied on the host by zeroing the fixed-patch data of dropped
patches (ff=0 -> dot=0 -> contribution 0); the host also computes the
tanh, keep count and final division over the 3375 per-patch dot products
the device returns, so the device computes exactly the two fp8 GEMMs and
the per-patch feature dots.

Device design (v2):
  * K packed to 39 tiles of 128 (4992 >= 4913); 19 DoubleRow pairs
    (tiles 0..37, K=256/instruction) + one plain fp8 matmul on tile 38.
    No device-side memsets of data tiles.
  * All data chunks ride the sync HWDGE ring in consumption order (a
    single busy ring sustains ~400GB/s; dual-ring data measured ~40%
    slower); weights + mp tile 38 ride the scalar ring alone.
  * The PE clock-gate needs ~3us of continuous execution to reach full
    rate and any gap >~200ns resets the ramp; junk warmup matmuls
    bridge from engine start to first-chunk arrival and a tuned number
    of junk fillers after each chunk group keep the PE gapless while
    the stream (the true bottleneck) catches up.
  * mp tile 38 rides a tiny early DMA and its matmul runs mid-stream, so
    the final accumulation step is DR pair 18, split into two column
    halves with per-half stop: TT/dot for half A overlap half B's matmul
    and the out DMA follows ~1.3us after the last data lands.
"""

import numpy as np
import ml_dtypes

import concourse.bacc as bacc
import concourse.mybir as mybir
import concourse.tile as tile
from concourse.bass_utils import run_bass_kernel_spmd

PATCH = 17
NPP = 15
N_TOT = NPP**3            # 3375 patches
P3 = PATCH**3             # 4913 elems per patch
F = 64                    # feature dim
N_CORES = 8
NP = 422                  # real patches per core (8*422 = 3376 = 3375 + 1)
NP2 = 432                 # padded to a multiple of 16 (DoubleRow AP step)
KT = 39                   # K tiles of 128 (4913 padded to 4992)
KPAD = KT * 128           # 4992
LAST_T = KT - 1           # tile 38: plain (non-DR) matmul
PAIRS = 19                # DoubleRow pairs (tiles 0..37)
NH = NP2 // 2             # half width for the tail split (216)
ZERO_THRESH = 0.15

BF16 = mybir.dt.bfloat16
F32 = mybir.dt.float32
DT = mybir.dt.float8e4
NP_DT = ml_dtypes.float8_e4m3
DR = mybir.MatmulPerfMode.DoubleRow

WFA_T = 14                # tiles in the first (small) wf piece
N_WARMUP = 9              # junk MMs bridging engine-start -> first chunk

# chunk schedule: (lo, hi) tile ranges per volume.  All data rides the
# sync HWDGE ring in consumption order (a single busy ring sustains
# ~400GB/s; splitting data across both rings measured ~40% slower), with
# weights + mp tile 38 alone on the scalar ring.  First and last chunks
# are small so the pipeline ramps fast and the tail lag is minimal.
KL = P3 - 38 * 128        # real rows in K tile 38 (49)

FP_CHUNKS = [(0, 4), (4, 12), (12, 20), (20, 28), (28, 38)]
FP_RINGS = ["s", "s", "s", "s", "s"]
FP_FILL = [0, 0, 0, 0, 0]          # junk fillers after each chunk's MMs
MP_CHUNKS = [(0, 8), (8, 16), (16, 24), (24, 32), (32, 36), (36, 38)]
MP_RINGS = ["s", "s", "s", "s", "s", "s"]
MP_FILL = [0, 0, 0, 0, 5, 0]       # dep-fillers bridge to the tail chunk

_COMPILED = None  # cache so repeat kernel() calls reuse the program


def _build_nc():
    nc = bacc.Bacc("TRN2", target_bir_lowering=False, debug=False)

    fpt_d = nc.dram_tensor("fpt", [128, KT, NP2], DT, kind="ExternalInput")
    mpt_d = nc.dram_tensor("mpt", [128, KT, NP2], DT, kind="ExternalInput")
    wf_d = nc.dram_tensor("wf", [128, KT, F], DT, kind="ExternalInput")
    wm_d = nc.dram_tensor("wm", [128, KT, F], DT, kind="ExternalInput")
    out_d = nc.dram_tensor("out", [1, NP2], BF16, kind="ExternalOutput")

    with tile.TileContext(nc) as tc:
        with (
            tc.tile_pool(name="weights", bufs=1) as wpool,
            tc.tile_pool(name="fdata", bufs=len(FP_CHUNKS)) as fpool,
            tc.tile_pool(name="mdata", bufs=len(MP_CHUNKS)) as mpool,
            tc.tile_pool(name="tdata", bufs=1) as tpool,
            tc.tile_pool(name="small", bufs=1) as spool,
            tc.tile_pool(name="psum", bufs=1, space="PSUM") as ppool,
        ):
            ring = {"s": nc.sync, "a": nc.scalar}

            # head of each ring: wfa (scalar) and fp chunk 0 (sync) so the
            # first real matmuls gate only on small transfers
            wfa = wpool.tile([128, WFA_T, F], DT, tag="wfa")
            nc.scalar.dma_start(wfa[:], wf_d.ap()[:, :WFA_T, :])
            fch = []
            c = fpool.tile(
                [128, FP_CHUNKS[0][1] - FP_CHUNKS[0][0], NP2], DT, tag="fp"
            )
            nc.sync.dma_start(c[:], fpt_d.ap()[:, FP_CHUNKS[0][0]:FP_CHUNKS[0][1], :])
            fch.append(c)
            c = fpool.tile(
                [128, FP_CHUNKS[1][1] - FP_CHUNKS[1][0], NP2], DT, tag="fp"
            )
            nc.sync.dma_start(c[:], fpt_d.ap()[:, FP_CHUNKS[1][0]:FP_CHUNKS[1][1], :])
            fch.append(c)

            # K tile 38 holds only 49 real rows; both volumes' slices ride
            # tiny early DMAs on the scalar ring and their (partial-K plain)
            # matmuls run mid-stream, off the tail's critical path
            fp_l = tpool.tile([KL, 1, NP2], DT, tag="fp_last")
            nc.scalar.dma_start(fp_l[:], fpt_d.ap()[:KL, LAST_T:, :])
            mp_l = tpool.tile([KL, 1, NP2], DT, tag="mp_last")
            nc.scalar.dma_start(mp_l[:], mpt_d.ap()[:KL, LAST_T:, :])

            junk = spool.tile([128, 2, 256], DT, tag="junk")
            nc.vector.memset(junk[:], 0.0)
            ones_bf = spool.tile([F, 1], BF16, tag="ones_bf")
            nc.vector.memset(ones_bf[:], 1.0)

            ps_ff = ppool.tile([F, NP2], F32, tag="ff")
            ps_mf = ppool.tile([F, NP2], F32, tag="mf")
            ps_warm = ppool.tile([F, 256], F32, tag="warm")
            ps_dot_a = ppool.tile([1, NH], F32, tag="dotA")
            ps_dot_b = ppool.tile([1, NH], F32, tag="dotB")

            def junk_mm(src=None):
                # keeps the PE clock-gate ramped while waiting on the stream.
                # Fillers read their chunk tile (src) so the tile scheduler
                # cannot hoist them ahead of the chunk's DMA (dep-free junk
                # gets bunched at the program start, destroying the pacing).
                nc.tensor.matmul(
                    ps_warm[:],
                    lhsT=junk[:, :, :F],
                    rhs=junk[:] if src is None else src[:, 0:2, :256],
                    start=True,
                    stop=True,
                    perf_mode=DR,
                )

            for _ in range(N_WARMUP):
                junk_mm()

            wfb = wpool.tile([128, KT - WFA_T, F], DT, tag="wfb")
            nc.scalar.dma_start(wfb[:], wf_d.ap()[:, WFA_T:, :])

            def wf_pair(t):
                if 2 * t + 1 < WFA_T:
                    return wfa[:, 2 * t:2 * t + 2, :]
                o = 2 * t - WFA_T
                return wfb[:, o:o + 2, :]

            def wf_last():
                return wfb[:, LAST_T - WFA_T:LAST_T - WFA_T + 1, :]

            def stream(chunks, rings, fills, dram, pool, tag, psum, pair_w,
                       last_w, pre=None, first_two=None, tail_halves=False):
                tiles = first_two if first_two else []
                t = 0
                for ci, (lo, hi) in enumerate(chunks):
                    if ci >= len(tiles):
                        ch = pool.tile([128, hi - lo, NP2], DT, tag=tag)
                        ring[rings[ci]].dma_start(
                            ch[:], dram.ap()[:, lo:hi, :]
                        )
                        tiles.append(ch)
                    ch = tiles[ci]
                    if pre and ci in pre:
                        pre[ci]()
                    last_chunk = ci == len(chunks) - 1
                    while 2 * t + 1 < hi:
                        rel = 2 * t - lo
                        if tail_halves and last_chunk and 2 * t == hi - 2:
                            # final DR pair split into column halves with
                            # per-half stop so the tail pipeline starts
                            # after a half-width matmul
                            for sl in (slice(0, NH), slice(NH, NP2)):
                                nc.tensor.matmul(
                                    psum[:, sl],
                                    lhsT=pair_w(t),
                                    rhs=ch[:, rel:rel + 2, sl],
                                    start=False,
                                    stop=True,
                                    perf_mode=DR,
                                    skip_group_check=True,
                                )
                        else:
                            nc.tensor.matmul(
                                psum[:],
                                lhsT=pair_w(t),
                                rhs=ch[:, rel:rel + 2, :],
                                start=(t == 0),
                                stop=False,
                                perf_mode=DR,
                            )
                        t += 1
                    if not tail_halves and hi == KT:
                        # plain (K=128) matmul on the odd last tile
                        nc.tensor.matmul(
                            psum[:],
                            lhsT=last_w(),
                            rhs=ch[:, hi - 1 - lo:hi - lo, :],
                            start=False,
                            stop=True,
                        )
                    for _ in range(fills[ci]):
                        junk_mm(src=ch)

            # ---- phase 1: fixed volume ----
            stream(FP_CHUNKS, FP_RINGS, FP_FILL, fpt_d, fpool, "fp", ps_ff,
                   wf_pair, wf_last, first_two=fch)
            # tile-38 closes the ff accumulation (data arrived long ago)
            nc.tensor.matmul(
                ps_ff[:],
                lhsT=wfb[:KL, LAST_T - WFA_T:LAST_T - WFA_T + 1, :],
                rhs=fp_l[:, 0:1, :],
                start=False,
                stop=True,
            )

            # stage ff out of PSUM on the DVE (TT can read only one PSUM
            # operand); runs mid-stream, off the critical path
            ff_sb = spool.tile([F, NP2], F32, tag="ff_sb")
            nc.vector.tensor_scalar(
                out=ff_sb[:], in0=ps_ff[:], scalar1=0.0, scalar2=None,
                op0=mybir.AluOpType.add,
            )

            # ---- phase 2: moving volume ----
            wm_sb = wpool.tile([128, KT, F], DT, tag="wm")
            nc.scalar.dma_start(wm_sb[:], wm_d.ap())

            def mp_t38():
                nc.tensor.matmul(
                    ps_mf[:],
                    lhsT=wm_sb[:KL, LAST_T:LAST_T + 1, :],
                    rhs=mp_l[:, 0:1, :],
                    start=False,
                    stop=False,
                    skip_group_check=True,
                )

            stream(MP_CHUNKS, MP_RINGS, MP_FILL, mpt_d, mpool, "mp", ps_mf,
                   lambda t: wm_sb[:, 2 * t:2 * t + 2, :], None,
                   pre={1: mp_t38}, tail_halves=True)

            # ---- tail: prod -> dot -> copy -> out DMA, two-half pipeline
            prod = spool.tile([F, NP2], BF16, tag="prod")
            outs = spool.tile([1, NP2], BF16, tag="outs")
            ps_dot = [ps_dot_a, ps_dot_b]
            HALVES = (slice(0, NH), slice(NH, NP2))
            for sl in HALVES:
                nc.vector.tensor_tensor(
                    out=prod[:, sl], in0=ff_sb[:, sl], in1=ps_mf[:, sl],
                    op=mybir.AluOpType.mult,
                )
            for h, sl in enumerate(HALVES):
                nc.tensor.matmul(
                    ps_dot[h][:], lhsT=ones_bf[:], rhs=prod[:, sl],
                    start=True, stop=True,
                )
            for h, sl in enumerate(HALVES):
                nc.vector.tensor_scalar(
                    out=outs[:, sl], in0=ps_dot[h][:], scalar1=0.0,
                    scalar2=None, op0=mybir.AluOpType.add,
                )
            nc.scalar.dma_start(out_d.ap(), outs[:])

    nc.compile()
    return nc


def _get_nc():
    global _COMPILED
    if _COMPILED is None:
        _COMPILED = _build_nc()
    return _COMPILED


def _prep_inputs(fixed, moving, Wf, Wm):
    """Host-side prep: patch-extract to K-major fp8, apply keep mask, pack.

    Returns (per-core input maps, keep mask, keep_count).
    """

    def vol_to_kmajor(vol):
        # vol [255,255,255] f32 -> [4913, 3375] f32 (K-major patches)
        x = vol.reshape(NPP, PATCH, NPP, PATCH, NPP, PATCH)
        x = x.transpose(1, 3, 5, 0, 2, 4)  # [17,17,17, 15,15,15]
        return np.ascontiguousarray(x).reshape(P3, N_TOT)

    def pad_shard(km8):
        shards = []
        for c in range(N_CORES):
            cols = km8[:, c * NP:min((c + 1) * NP, N_TOT)]
            sh = np.zeros((KPAD, NP2), dtype=NP_DT)
            sh[:P3, :cols.shape[1]] = cols
            # [KPAD, NP2] -> [128, KT, NP2]: partition p holds K rows
            # {t*128+p}, contiguous t-major per partition
            a = sh.reshape(KT, 128, NP2).transpose(1, 0, 2)
            shards.append(np.ascontiguousarray(a))
        return shards

    def pack_w(W):
        wp = np.zeros((KPAD, F), dtype=np.float32)
        wp[:P3] = W
        wp = wp.reshape(KT, 128, F).transpose(1, 0, 2)
        return np.ascontiguousarray(wp.astype(NP_DT))

    fkm = vol_to_kmajor(np.asarray(fixed)[0, 0])    # f32, exact
    mkm = vol_to_kmajor(np.asarray(moving)[0, 0])

    # reference keep mask computed from the exact f32 fixed patches
    zero_cnt = (fkm == 0).sum(axis=0)               # [3375]
    keep = zero_cnt <= ZERO_THRESH * P3
    keep_count = int(keep.sum())

    fkm8 = fkm.astype(NP_DT)
    fkm8[:, ~keep] = 0  # dropped patches contribute exactly 0 to the sum
    mkm8 = mkm.astype(NP_DT)

    fp_shards = pad_shard(fkm8)
    mp_shards = pad_shard(mkm8)
    wf_p = pack_w(np.asarray(Wf))
    wm_p = pack_w(np.asarray(Wm))

    in_maps = [
        {"fpt": fp_shards[c], "mpt": mp_shards[c], "wf": wf_p, "wm": wm_p}
        for c in range(N_CORES)
    ]
    return in_maps, keep, keep_count


def _run(inputs, trace=False, **kwargs):
    nc = _get_nc()
    in_maps, keep, keep_count = _prep_inputs(
        inputs["fixed"], inputs["moving"], inputs["Wf"], inputs["Wm"]
    )
    res = run_bass_kernel_spmd(nc, in_maps, list(range(N_CORES)), trace=trace, **kwargs)
    dots = np.concatenate(
        [np.asarray(r["out"], dtype=np.float64).reshape(NP2)[:NP] for r in res.results]
    )[:N_TOT]
    s = float(np.sum(np.tanh(dots) * keep))
    value = np.float32(s / max(keep_count, 1.0))
    return np.asarray(value, dtype=np.float32), res


def kernel(**inputs) -> np.ndarray:
    value, _ = _run(inputs, trace=False)
    return value
